# revision 1
# baseline (speedup 1.0000x reference)
"""Trainium2 Bass kernel for CorefContrastiveLoss.

loss = mean_i [ -sum_{j!=i} lbl[i,j] * log_softmax_j(sim[i,j]) ]
sim = (x_hat @ x_hat.T) / T,  x_hat = emb / max(||emb||, eps)

Strategy (8 cores, no collectives):
  * Each core receives the FULL embedding matrix *rolled* so that its
    1024-row block sits at rows 0..1023, plus its (column-rolled) block of
    cr_labels.  This makes the program core-agnostic (pure SPMD).
  * On device: normalize rows in fp32, scale by sqrt(1/T), cast to bf16,
    DMA-xbar-transpose into x_hat^T chunks, then a bf16 GEMM computes the
    1024x8192 sim block.  Evictions are fused:
      - ACT Exp(sim - 5) with accum_out -> Z row-sum partials
      - DVE tensor_tensor_reduce(lbl * sim) -> A partials
      - DVE tensor_reduce(lbl) -> L partials
      - identity-masked ttr -> diagonal sim / lbl values
  * Host combines partials in float64:
      loss_i = -(A - lbl_d*sim_d) + (L - lbl_d) * (5 + log(Z - exp(sim_d-5)))
"""

import numpy as np

import concourse.bass as bass  # noqa: F401  (kept for API parity)
import concourse.mybir as mybir
import concourse.tile as tile
from concourse import bacc
from concourse.bass_utils import run_bass_kernel_spmd
from concourse.masks import make_identity

# Problem geometry (hardcoded for the graded problem).
N = 8192          # mentions
D = 1024          # embedding dim
C = 8             # cores
P = 128           # partitions
NTW = 512         # sim column-tile width (one PSUM bank of fp32)
TEMP = 0.2
SHIFT = 1.0 / TEMP          # 5.0 == max possible |sim| value; exp shift
RSQRT_T = (1.0 / TEMP) ** 0.5
EPS = 1e-8

F32 = mybir.dt.float32
BF16 = mybir.dt.bfloat16
MULT = mybir.AluOpType.mult
ADD = mybir.AluOpType.add


def _pin_act_table_set():
    """Make natural_log_exp_and_others the only set claiming the funcs we
    use, so the act-table-load pass emits a single table load instead of
    thrashing between per-function sets (~2.7us per reload on HW).  Dict
    order (= act_func_set_id) is preserved, only membership is edited."""
    from concourse import bacc as _bacc

    if getattr(_bacc, "_act_tables_pinned", False):
        return
    _orig = _bacc.get_activation_tables
    mine = {
        mybir.ActivationFunctionType.Exp,
        mybir.ActivationFunctionType.Ln,
        mybir.ActivationFunctionType.Square,
        mybir.ActivationFunctionType.Copy,
        mybir.ActivationFunctionType.Identity,
    }

    def _patched(arch):
        t = _orig(arch)
        if "natural_log_exp_and_others" in t and mine <= t[
            "natural_log_exp_and_others"
        ]:
            for name in t:
                if name != "natural_log_exp_and_others":
                    t[name] = t[name] - mine
        return t

    _bacc.get_activation_tables = _patched
    _bacc._act_tables_pinned = True


_pin_act_table_set()


def build_nc(n=N, d=D, c=C):
    """Build + compile the per-core (SPMD) Bass program."""
    from contextlib import ExitStack

    nb = n // c            # rows per core
    mt = nb // P           # m-tiles per core
    kt = d // P            # contraction chunks
    nnt = n // NTW         # sim column tiles

    nc = bacc.Bacc("TRN2", target_bir_lowering=False, debug=False, num_devices=c)

    emb = nc.dram_tensor("emb", [n, d], F32, kind="ExternalInput")
    lbl = nc.dram_tensor("lbl", [nb, n], F32, kind="ExternalInput")
    zp_d = nc.dram_tensor("zp", [P, mt * nnt], F32, kind="ExternalOutput")
    ap_d = nc.dram_tensor("apar", [P, mt * nnt], F32, kind="ExternalOutput")
    lp_d = nc.dram_tensor("lpar", [P, mt * nnt], F32, kind="ExternalOutput")
    sd_d = nc.dram_tensor("sd", [P, mt], F32, kind="ExternalOutput")
    ld_d = nc.dram_tensor("ld", [P, mt], F32, kind="ExternalOutput")

    with tile.TileContext(nc) as tc, ExitStack() as ctx:
        singles = ctx.enter_context(tc.tile_pool(name="singles", bufs=1))
        xt_pool = ctx.enter_context(tc.tile_pool(name="xt", bufs=1))
        e_pool = ctx.enter_context(tc.tile_pool(name="ein", bufs=3))
        sq_pool = ctx.enter_context(tc.tile_pool(name="sq", bufs=2))
        nrm_pool = ctx.enter_context(tc.tile_pool(name="nrm", bufs=4))
        xh_pool = ctx.enter_context(tc.tile_pool(name="xh", bufs=3))
        lbl_pool = ctx.enter_context(tc.tile_pool(name="lblp", bufs=4))
        ex_pool = ctx.enter_context(tc.tile_pool(name="ex", bufs=3))
        tt_pool = ctx.enter_context(tc.tile_pool(name="tt", bufs=3))
        d_pool = ctx.enter_context(tc.tile_pool(name="dg", bufs=2))
        psum_pool = ctx.enter_context(tc.tile_pool(name="psum", bufs=8, space="PSUM"))

        ident = singles.tile([P, P], F32, tag="ident")
        make_identity(nc, ident)

        bias_t = singles.tile([P, 1], F32, tag="bias_t")
        nc.vector.memset(bias_t[:, :], -SHIFT)
        # bias for inv-norm: exp(-0.5*ln(ss) + 0.5*ln(1/T)) = sqrt(1/T)/||e||
        # (keeps every ACT func in the natural_log_exp_and_others table set)
        bias_i = singles.tile([P, 1], F32, tag="bias_i")
        nc.vector.memset(bias_i[:, :], 0.5 * float(np.log(1.0 / TEMP)))

        zp_s = singles.tile([P, mt * nnt], F32, tag="zp_s")
        ap_s = singles.tile([P, mt * nnt], F32, tag="ap_s")
        lp_s = singles.tile([P, mt * nnt], F32, tag="lp_s")
        sd_s = singles.tile([P, mt], F32, tag="sd_s")
        ld_s = singles.tile([P, mt], F32, tag="ld_s")

        # x_hat^T: [p, k, col] = x_hat[col, k*128+p].  One tile; Tile tracks
        # subtile deps so stage-2 matmuls start as slices become ready.
        xt = xt_pool.tile([P, kt, n], BF16, tag="xt", name="xt")

        # ---- stage 1: normalize + cast + transpose ----
        for t in range(n // P):
            et = e_pool.tile([P, d], F32, tag="et")
            nc.sync.dma_start(out=et[:, :], in_=emb[t * P:(t + 1) * P, :])

            sq = sq_pool.tile([P, d], BF16, tag="sqs")
            ss = nrm_pool.tile([P, 1], F32, tag="ss")
            nc.scalar.activation(
                out=sq[:, :], in_=et[:, :],
                func=mybir.ActivationFunctionType.Square,
                accum_out=ss[:, :],
            )
            lnv = nrm_pool.tile([P, 1], F32, tag="lnv")
            nc.scalar.activation(
                out=lnv[:, :], in_=ss[:, :],
                func=mybir.ActivationFunctionType.Ln,
            )
            inv = nrm_pool.tile([P, 1], F32, tag="inv")
            nc.scalar.activation(
                out=inv[:, :], in_=lnv[:, :],
                func=mybir.ActivationFunctionType.Exp,
                bias=bias_i[:, :], scale=-0.5,
            )

            xh = xh_pool.tile([P, d], BF16, tag="xh")
            nc.gpsimd.tensor_scalar(
                xh[:, :], et[:, :], inv[:, :], None, MULT
            )
            # one xbar transpose per E-tile: [128, d] -> [128, kt, 128]
            nc.scalar.dma_start_transpose(
                out=xt[:, :, t * P:(t + 1) * P],
                in_=xh[:, :],
            )

        # ---- stage 2: GEMM + fused evictions ----
        for m in range(mt):
            ntb, offb = m // 4, (m % 4) * P
            lbB = None
            for nt in range(nnt):
                lbg = min(4, nnt)
                if nt % lbg == 0:
                    # batched label load: lbg column tiles per DMA
                    lbB = lbl_pool.tile([P, lbg * NTW], F32, tag="lbB",
                                        name="lbB")
                    nc.sync.dma_start(
                        out=lbB[:, :],
                        in_=lbl[m * P:(m + 1) * P,
                                nt * NTW:(nt + lbg) * NTW],
                    )
                lb = lbB[:, (nt % lbg) * NTW:(nt % lbg + 1) * NTW]
                ps = psum_pool.tile([P, NTW], F32, tag="ps")
                for k in range(kt):
                    nc.tensor.matmul(
                        ps[:, :],
                        lhsT=xt[:, k, m * P:(m + 1) * P],
                        rhs=xt[:, k, nt * NTW:(nt + 1) * NTW],
                        start=(k == 0),
                        stop=(k == kt - 1),
                    )
                idx = m * nnt + nt
                ex = ex_pool.tile([P, NTW], BF16, tag="ex")
                nc.scalar.activation(
                    out=ex[:, :], in_=ps[:, :],
                    func=mybir.ActivationFunctionType.Exp,
                    bias=bias_t[:, :],
                    accum_out=zp_s[:, idx:idx + 1],
                )
                # A partial: lbl * sim, then row-reduce (DVE)
                tt_t = tt_pool.tile([P, NTW], F32, tag="tts")
                nc.vector.tensor_tensor(
                    out=tt_t[:, :], in0=ps[:, :], in1=lb[:, :], op=MULT
                )
                nc.vector.tensor_reduce(
                    out=ap_s[:, idx:idx + 1], in_=tt_t[:, :],
                    axis=mybir.AxisListType.X, op=ADD,
                )
                # L partial: row-reduce of labels; split DVE/ACT to balance
                if nt % 2 == 0:
                    nc.vector.tensor_reduce(
                        out=lp_s[:, idx:idx + 1], in_=lb[:, :],
                        axis=mybir.AxisListType.X, op=ADD,
                    )
                else:
                    lcp = ex_pool.tile([P, NTW], BF16, tag="lcp")
                    nc.scalar.activation(
                        out=lcp[:, :], in_=lb[:, :],
                        func=mybir.ActivationFunctionType.Copy,
                        accum_out=lp_s[:, idx:idx + 1],
                    )
                if nt == ntb:
                    # diagonal of this row-block lives in this column tile
                    dsc = d_pool.tile([P, P], F32, tag="dsc")
                    nc.vector.tensor_tensor(
                        out=dsc[:, :], in0=ps[:, offb:offb + P],
                        in1=ident[:, :], op=MULT,
                    )
                    nc.vector.tensor_reduce(
                        out=sd_s[:, m:m + 1], in_=dsc[:, :],
                        axis=mybir.AxisListType.X, op=ADD,
                    )
                    dsc2 = d_pool.tile([P, P], F32, tag="dsc2")
                    nc.vector.tensor_tensor(
                        out=dsc2[:, :], in0=lb[:, offb:offb + P],
                        in1=ident[:, :], op=MULT,
                    )
                    nc.vector.tensor_reduce(
                        out=ld_s[:, m:m + 1], in_=dsc2[:, :],
                        axis=mybir.AxisListType.X, op=ADD,
                    )

        nc.sync.dma_start(out=zp_d[:, :], in_=zp_s[:, :])
        nc.sync.dma_start(out=ap_d[:, :], in_=ap_s[:, :])
        nc.sync.dma_start(out=lp_d[:, :], in_=lp_s[:, :])
        nc.sync.dma_start(out=sd_d[:, :], in_=sd_s[:, :])
        nc.sync.dma_start(out=ld_d[:, :], in_=ld_s[:, :])

    nc.compile()
    return nc


def shard_inputs(mention_embs, cr_labels, n=N, c=C):
    """Per-core input maps: rolled full embeddings + rolled label row-block."""
    nb = n // c
    emb = np.ascontiguousarray(mention_embs, dtype=np.float32)
    in_maps = []
    for ci in range(c):
        emb_c = np.roll(emb, -ci * nb, axis=0)
        lbl_c = np.roll(
            np.ascontiguousarray(cr_labels[ci * nb:(ci + 1) * nb, :],
                                 dtype=np.float32),
            -ci * nb, axis=1,
        )
        in_maps.append({"emb": np.ascontiguousarray(emb_c),
                        "lbl": np.ascontiguousarray(lbl_c)})
    return in_maps


def combine(results, n=N, c=C):
    """Host-side float64 combine of per-core partial stats -> scalar loss."""
    nb = n // c
    mt = nb // P
    nnt = n // NTW
    total = 0.0
    for r in results:
        z = r["zp"].astype(np.float64).reshape(P, mt, nnt).sum(axis=-1)
        a = r["apar"].astype(np.float64).reshape(P, mt, nnt).sum(axis=-1)
        ll = r["lpar"].astype(np.float64).reshape(P, mt, nnt).sum(axis=-1)
        sd = r["sd"].astype(np.float64)
        ld = r["ld"].astype(np.float64)
        z_off = z - np.exp(sd - SHIFT)
        lse = SHIFT + np.log(z_off)
        loss_rows = -(a - ld * sd) + (ll - ld) * lse
        total += loss_rows.sum()
    return np.float32(total / n)


_NC_CACHE = {}


def _get_nc():
    if "nc" not in _NC_CACHE:
        _NC_CACHE["nc"] = build_nc()
    return _NC_CACHE["nc"]


def kernel(mention_embs, cr_labels):
    nc = _get_nc()
    in_maps = shard_inputs(mention_embs, cr_labels)
    res = run_bass_kernel_spmd(nc, in_maps, list(range(C)))
    return combine(res.results)

